# revision 1
# baseline (speedup 1.0000x reference)
"""BatchHardTripletLoss on 8 Trainium2 NeuronCores.

Strategy (data-parallel over the batch):
  - B=8192 anchors split as 1024 rows per core; every core holds the full
    embeddings (column side of the distance matrix).
  - The label mask and the per-column squared-norm term are folded into the
    matmul itself: with ``M'[i,j] = sq_j - 2*x_i.x_j + 1024*[l_i==l_j]``
    (computed as a feature matmul plus a one-hot matmul, then the sq_j row
    added during the reduction), we get
        hardest_pos^2 = max_j M'[i,j] + sq_i - 1024   (self gives exactly 0)
        hardest_neg^2 = min_j M'[i,j] + sq_i
    because the +1024 penalty lifts every same-label entry above all
    different-label entries (d2 <= ~500 << 1024), and sqrt is monotonic so
    min/max commute with it.
  - Each core reduces its 1024x8192 stripe to 1024 per-row losses
    relu(hp - hn + 1); the host masks invalid anchors (label counts) and
    takes the mean -- the cross-device reduction of the sharding hint.
"""

import sys

import numpy as np

if "/opt/trn_rl_repo" not in sys.path:
    sys.path.insert(0, "/opt/trn_rl_repo")

from concourse import bacc, bass, mybir, tile
from concourse.bass_utils import run_bass_kernel_spmd

B = 8192
D = 128
C = 128
N_CORES = 8
R = B // N_CORES          # rows per core
RT = R // 128             # row tiles per core (8)
CBW = 2048                # main-loop column superblock width
NCB = B // CBW            # 4 superblocks
PEN = 1024.0              # same-label penalty (> max possible d2)

F16 = mybir.dt.float16
F32 = mybir.dt.float32
ALU = mybir.AluOpType
ACTF = mybir.ActivationFunctionType

_NC_CACHE = {}


def _build_nc(maxw=CBW):
    nc = bacc.Bacc(None, target_bir_lowering=False)

    xt_d = nc.declare_dram_parameter("xt", [128, B], F16, isOutput=False)
    oh_d = nc.declare_dram_parameter("oh", [128, B], F16, isOutput=False)
    xts_d = nc.declare_dram_parameter("xts", [128, R], F16, isOutput=False)
    ohss_d = nc.declare_dram_parameter("ohss", [128, R], F16, isOutput=False)
    onesm_d = nc.declare_dram_parameter("onesm", [128, 128], F16, isOutput=False)
    out_d = nc.declare_dram_parameter("out_pr", [128, RT], F32, isOutput=True)

    with tile.TileContext(nc) as tc:
        with tc.tile_pool(name="const", bufs=1) as cp:
            XT = [cp.tile([128, CBW], F16, tag=f"xt{q}", name=f"xt{q}") for q in range(NCB)]
            OH = [cp.tile([128, CBW], F16, tag=f"oh{q}", name=f"oh{q}") for q in range(NCB)]
            XSQ = [cp.tile([128, CBW], F16, tag=f"xsq{q}", name=f"xsq{q}") for q in range(NCB)]
            AUG = [cp.tile([128, CBW], F16, tag=f"aug{q}", name=f"aug{q}") for q in range(NCB)]
            XTS = cp.tile([128, R], F16)
            OHSS = cp.tile([128, R], F16)
            ONESM = cp.tile([128, 128], F16)
            XLS = cp.tile([128, R], F16)    # -2 * x shard (lhsT chunk A)
            XSQS = cp.tile([128, R], F16)   # x^2 shard
            SQI = cp.tile([128, RT], F32)   # sq_i per shard row
            AMX = cp.tile([128, RT], F32)   # per-rowtile max
            AMN = cp.tile([128, RT], F32)   # per-rowtile min

            # block-0 data first (XSQ[0]/AUG[0] gate the main loop's start);
            # spread issue across engine queues so SP isn't a serial bottleneck
            nc.sync.dma_start(ONESM[:], onesm_d[:])
            nc.sync.dma_start(XT[0][:], xt_d[:, 0:CBW])
            nc.sync.dma_start(OH[0][:], oh_d[:, 0:CBW])
            nc.sync.dma_start(XTS[:], xts_d[:])
            nc.sync.dma_start(OHSS[:], ohss_d[:])
            for q in range(1, NCB):
                s = q * CBW
                nc.sync.dma_start(XT[q][:], xt_d[:, s : s + CBW])
                nc.sync.dma_start(OH[q][:], oh_d[:, s : s + CBW])

            # prep: scaled shard operand + elementwise squares
            nc.vector.tensor_scalar_mul(XLS[:], XTS[:], -2.0)
            nc.vector.tensor_tensor(XSQS[:], XTS[:], XTS[:], op=ALU.mult)
            # touch Sqrt early so its ACT table set loads off the critical tail
            WARM = cp.tile([128, 1], F32)
            WARMZ = cp.tile([128, 1], F32)
            nc.vector.memset(WARMZ[:], 0.0)
            nc.scalar.activation(WARM[:], ONESM[:, 0:1], ACTF.Sqrt, bias=WARMZ[:])
            for q in range(NCB):
                nc.vector.tensor_tensor(XSQ[q][:], XT[q][:], XT[q][:], op=ALU.mult)

            # main loop: psum[i, j] = -2 x_i.x_j + sq_j + 1024*[same label]
            # Columns are per-core permuted so every same-label column of this
            # core's rows lives in superblock 0 -> the max (hardest-positive)
            # reduction only scans cb==0; the min scans all blocks.
            ACCN = cp.tile([128, 512 * RT], F16)  # per-rt running min, 512 wide
            T0 = cp.tile([128, CBW * RT], F16)    # cb0 tiles kept for the
            #                                       deferred max folds
            # finalize tiles (written in two rt-groups of 4)
            HP2 = cp.tile([128, RT], F32)
            HN2 = cp.tile([128, RT], F32)
            HP = cp.tile([128, RT], F32)
            HN = cp.tile([128, RT], F32)
            TMPF = cp.tile([128, RT], F32)
            RCP = cp.tile([128, RT], F32)
            OUT = cp.tile([128, RT], F32)
            BNEG = cp.tile([128, 1], F32)
            BONE = cp.tile([128, 1], F32)
            BZRO = cp.tile([128, 1], F32)
            nc.vector.memset(BNEG[:], -PEN)
            nc.vector.memset(BONE[:], 1.0)
            nc.vector.memset(BZRO[:], 0.0)

            def finalize_group(g):
                # hp = sqrt(relu(amx + sq_i - 1024)), hn = sqrt(relu(amn + sq_i))
                # out = relu(hp - hn + 1); one Newton step refines each sqrt
                s = slice(g * 4, g * 4 + 4)
                nc.vector.tensor_tensor(TMPF[:, s], AMX[:, s], SQI[:, s], op=ALU.add)
                nc.scalar.activation(HP2[:, s], TMPF[:, s], ACTF.Relu, bias=BNEG[:])
                nc.vector.tensor_tensor(TMPF[:, s], AMN[:, s], SQI[:, s], op=ALU.add)
                nc.scalar.activation(HN2[:, s], TMPF[:, s], ACTF.Relu, bias=BZRO[:])
                nc.scalar.activation(HP[:, s], HP2[:, s], ACTF.Sqrt, bias=BZRO[:])
                nc.scalar.activation(HN[:, s], HN2[:, s], ACTF.Sqrt, bias=BZRO[:])
                for (Hs, Vs) in ((HP, HP2), (HN, HN2)):
                    nc.vector.tensor_scalar_add(TMPF[:, s], Hs[:, s], 1.0e-12)
                    nc.vector.reciprocal(RCP[:, s], TMPF[:, s])
                    nc.vector.tensor_tensor(TMPF[:, s], Vs[:, s], RCP[:, s], op=ALU.mult)
                    nc.vector.tensor_tensor(Hs[:, s], Hs[:, s], TMPF[:, s], op=ALU.add)
                    nc.vector.tensor_scalar_mul(Hs[:, s], Hs[:, s], 0.5)
                nc.vector.tensor_tensor(TMPF[:, s], HP[:, s], HN[:, s], op=ALU.subtract)
                nc.scalar.activation(OUT[:, s], TMPF[:, s], ACTF.Relu, bias=BONE[:])
            with (
                tc.tile_pool(name="mpsum", bufs=2, space=bass.MemorySpace.PSUM) as mp,
                tc.tile_pool(name="tbuf", bufs=7) as tp,
                tc.tile_pool(name="fold", bufs=10) as fpool,
            ):
                def build_aug(q, direct=False):
                    # AUG[c,j] = sum_f XSQ[f,j] + 1024*OH[c,j]: the all-ones
                    # stationary broadcasts column sums to every partition;
                    # OH arrives pre-scaled by 1024 and is added on DVE.
                    # direct=True skips the ACT psum->f16 hop (ramp builds,
                    # where DVE is idle and ACT latency gates the first tiles).
                    ps = mp.tile([128, CBW], F32, tag="ps", name=f"augps{q}")
                    for k in range(CBW // 512):
                        sl = slice(k * 512, (k + 1) * 512)
                        nc.tensor.matmul(
                            ps[:, sl], ONESM[:], XSQ[q][:, sl],
                            start=True, stop=True,
                        )
                    if direct:
                        nc.vector.tensor_tensor(AUG[q][:], ps[:], OH[q][:], op=ALU.add)
                        return
                    SQB = tp.tile([128, CBW], F16, tag="tt", name=f"sqb{q}")
                    nc.scalar.activation(SQB[:], ps[:], ACTF.Copy)
                    nc.vector.tensor_tensor(AUG[q][:], SQB[:], OH[q][:], op=ALU.add)

                def build_sqi():
                    # SQI[p, t] = sq of shard row 128 t + p
                    pi = mp.tile([128, CBW], F32, tag="ps", name="sqips")
                    for t in range(RT):
                        nc.tensor.matmul(
                            pi[:, t : t + 1],
                            XSQS[:, t * 128 : (t + 1) * 128], ONESM[:, 0:1],
                            start=True, stop=True,
                        )
                    nc.scalar.activation(SQI[:], pi[:, 0:RT], ACTF.Copy)

                def do_max_fold(rt):
                    # deferred max path: only the first maxw columns of the
                    # own-label superblock can win (fillers are value-excluded)
                    h1, h2 = maxw // 2, maxw // 4
                    t = T0[:, rt * CBW : (rt + 1) * CBW]
                    G1 = fpool.tile([128, h1], F16, tag="g1")
                    G2 = fpool.tile([128, h2], F16, tag="g2")
                    nc.vector.tensor_tensor(
                        G1[:], t[:, 0:h1], t[:, h1 : 2 * h1], op=ALU.max
                    )
                    nc.vector.tensor_tensor(
                        G2[:], G1[:, 0:h2], G1[:, h2 : 2 * h2], op=ALU.max
                    )
                    nc.vector.tensor_reduce(
                        AMX[:, rt : rt + 1], G2[:],
                        axis=mybir.AxisListType.X, op=ALU.max,
                    )

                build_aug(0)
                build_sqi()   # fills the PE/psum slack while AUG[0] finishes
                build_aug(1)  # ramp is DMA-gated anyway; psum slots are free
                for cb in range(NCB):
                    for rt in range(RT):
                        if rt == 2 and cb + 2 < NCB:
                            build_aug(cb + 2)   # prefetch next-next superblock
                        lA = XLS[:, rt * 128 : (rt + 1) * 128]
                        lB = OHSS[:, rt * 128 : (rt + 1) * 128]
                        ps = mp.tile([128, CBW], F32, tag="ps")
                        for k in range(CBW // 512):
                            sl = slice(k * 512, (k + 1) * 512)
                            nc.tensor.matmul(
                                ps[:, sl], lA, XT[cb][:, sl], start=True, stop=False
                            )
                        for k in range(CBW // 512):
                            sl = slice(k * 512, (k + 1) * 512)
                            nc.tensor.matmul(
                                ps[:, sl], lB, AUG[cb][:, sl], start=False, stop=True
                            )
                        if cb == 3 and rt == RT - 1:
                            # terminal tile: reduce straight from PSUM so the
                            # tail skips the copy->fold->final serial chain
                            PD = fpool.tile([128, 1], F32, tag="pd")
                            TRD = fpool.tile([128, 1], F32, tag="trd")
                            nc.vector.tensor_reduce(
                                PD[:], ps[:], axis=mybir.AxisListType.X, op=ALU.min
                            )
                            nc.vector.tensor_reduce(
                                TRD[:], ACCN[:, rt * 512 : rt * 512 + 512],
                                axis=mybir.AxisListType.X, op=ALU.min,
                            )
                            nc.vector.tensor_tensor(
                                AMN[:, rt : rt + 1], TRD[:], PD[:], op=ALU.min
                            )
                            finalize_group(1)
                            continue
                        if cb == 0:
                            T = T0[:, rt * CBW : (rt + 1) * CBW]
                        else:
                            Tt = tp.tile([128, CBW], F16, tag="tt")
                            T = Tt[:]
                        nc.scalar.activation(T, ps[:], ACTF.Copy)
                        # min path: fold 2048 -> 1024 -> 512, then accumulate
                        F1 = fpool.tile([128, 1024], F16, tag="f1")
                        F2 = fpool.tile([128, 512], F16, tag="f2")
                        nc.vector.tensor_tensor(
                            F1[:], T[:, 0:1024], T[:, 1024:2048], op=ALU.min
                        )
                        an = ACCN[:, rt * 512 : rt * 512 + 512]
                        if cb == 0:
                            nc.vector.tensor_tensor(
                                an, F1[:, 0:512], F1[:, 512:1024], op=ALU.min
                            )
                        else:
                            nc.vector.tensor_tensor(
                                F2[:], F1[:, 0:512], F1[:, 512:1024], op=ALU.min
                            )
                            nc.vector.tensor_tensor(an, an, F2[:], op=ALU.min)
                        # interleave the deferred cb0 max folds into spare
                        # DVE slots of the cb1/cb2 sweeps
                        if cb == 1 and rt % 2 == 1:
                            do_max_fold(rt // 2)
                        elif cb == 2 and rt % 2 == 1:
                            do_max_fold(4 + rt // 2)
                        elif cb == 3:
                            # final per-rt min as soon as its last acc lands
                            nc.vector.tensor_reduce(
                                AMN[:, rt : rt + 1],
                                ACCN[:, rt * 512 : rt * 512 + 512],
                                axis=mybir.AxisListType.X, op=ALU.min,
                            )
                            if rt == 3:
                                finalize_group(0)
                            elif rt == 7:
                                finalize_group(1)

            nc.sync.dma_start(out_d[:], OUT[:])

    nc.compile()
    return nc


def _get_nc(maxw=CBW):
    if maxw not in _NC_CACHE:
        _NC_CACHE[maxw] = _build_nc(maxw)
    return _NC_CACHE[maxw]


def _prep_in_maps(embeddings, labels):
    """Sort rows by label (pure permutation; inverted on output) and give each
    core a column permutation that puts all same-label columns of its row
    shard inside the first CBW columns, so the max reduction scans one
    superblock. Returns (in_maps, labels, order)."""
    x = np.asarray(embeddings, dtype=np.float32)            # [B, D]
    lab = np.asarray(labels).astype(np.int64)               # [B]
    order = np.argsort(lab, kind="stable")
    lab_s = lab[order]
    xt = np.ascontiguousarray(x[order].T).astype(np.float16)  # [128, B] sorted
    onesm = np.ones((128, 128), dtype=np.float16)
    in_maps = []
    for m in range(N_CORES):
        sl = slice(m * R, (m + 1) * R)
        shard_labels = np.unique(lab_s[sl])
        own = np.flatnonzero(np.isin(lab_s, shard_labels))
        if len(own) > CBW:
            raise RuntimeError(
                f"own-label span {len(own)} exceeds superblock {CBW}"
            )
        rest = np.flatnonzero(~np.isin(lab_s, shard_labels))
        perm = np.concatenate([own, rest])
        lab_p = lab_s[perm]
        xt_m = np.ascontiguousarray(xt[:, perm])
        oh_m = (PEN * (lab_p[None, :] == np.arange(C)[:, None])).astype(np.float16)
        ohss = (lab_s[sl][None, :] == np.arange(C)[:, None]).astype(np.float16)
        in_maps.append({
            "_ownw": len(own),
            "xt": xt_m,
            "oh": oh_m,
            "xts": np.ascontiguousarray(xt[:, sl]),
            "ohss": ohss,
            "onesm": onesm,
        })
    return in_maps, lab, order


def run_cores(embeddings, labels, trace=False, **kw):
    in_maps, lab, order = _prep_in_maps(embeddings, labels)
    maxw = max(int(m.pop("_ownw")) for m in in_maps)
    maxw = min(CBW, max(512, -(-maxw // 512) * 512))
    nc = _get_nc(maxw)
    res = run_bass_kernel_spmd(nc, in_maps, list(range(N_CORES)), trace=trace, **kw)
    pr_sorted = np.concatenate(
        [np.asarray(r["out_pr"], np.float32).T.reshape(R) for r in res.results]
    )
    pr = np.empty(B, np.float32)
    pr[order] = pr_sorted
    counts = np.bincount(lab, minlength=C)
    valid = (counts[lab] >= 2) & (counts[lab] <= B - 1)
    nv = int(valid.sum())
    loss = float((pr * valid).sum() / nv) if nv > 0 else 0.0
    return np.float32(loss), res


def kernel(embeddings, labels):
    loss, _ = run_cores(embeddings, labels, trace=False)
    return loss

